# revision 1
# baseline (speedup 1.0000x reference)
"""Trainium2 Bass kernel for nn_DiffeqSolver (two-step Euler MLP-ODE).

Math (per trajectory n, time step i):
    f(y) = tanh(y@W1 + b1)@W2 + b2
    h_i = t_i / 2
    y1_i = y0 + h_i*f(y0)
    y2_i = y1_i + h_i*f(y1_i)
    out[n,i,:] = active[n,i] ? y2_i : 0      (active = any(mask[n,i,:] > 0))
    (t_i == 0 gives y2 == y0 exactly, so the reference's pos-branch is folded.)

Refactor used on device (feature-major, per core with T=1024 trajectories):
    g0  = f(y0)                      (once)
    z_i = W1^T y0^T + (h_i W1)^T g0^T + b1          (PE accumulate, 2 matmuls)
    u_i = tanh(z_i)                                  (ACT)
    y2_i^(chunk) = [Cb;y0]^T_chunk @ rhsD_i + (u_i_chunk)^T @ (h_i W2)
                   where Cb = g0 + b2, rhsD packs [h_i*I64; I64]
    -> y2 lands TRAJ-major in PSUM; one fused DVE op applies the active-mask
       and drains PSUM->SBUF; DMA out contiguous [128 traj, 8 steps x 64 lat].

Sharding: data-parallel over trajectories, 1024 per core x 8 cores.
"""

import numpy as np
from contextlib import ExitStack

import concourse.bass as bass
import concourse.bacc as bacc
import concourse.mybir as mybir
import concourse.tile as tile
import concourse.tile_sem_assignment as _tsa
from concourse.tile import add_dep_helper as _add_dep
from concourse.bass_utils import run_bass_kernel_spmd

# NOTE: excess sync waits (>1 per instruction) are legal here because
# Bacc.compile()'s generate_event_semaphores pass splits them; the nop
# pre-waits below just keep DMA instructions lean.

N_TRAJ, N_TIME, LAT, HID, DIM = 8192, 64, 64, 128, 64
NCORES = 8
T = N_TRAJ // NCORES          # 1024 trajectories per core
NCH = T // 128                # 8 chunks of 128 trajectories
SG = 8                        # steps per group
NG = N_TIME // SG             # 8 step groups
F32 = mybir.dt.float32
I32 = mybir.dt.int32
AF = mybir.ActivationFunctionType
OP = mybir.AluOpType

_cache = {}


def _emit(ctx, tc, nc, fp, ts, mask, W1, b1, W2, b2, out, ident, ipat_full):
    P = 128

    const = ctx.enter_context(tc.tile_pool(name="const", bufs=1))

    W1_sb = const.tile([LAT, HID], F32)
    nc.gpsimd.dma_start(W1_sb[:], W1[:])
    W2_sb = const.tile([HID, LAT], F32)
    nc.gpsimd.dma_start(W2_sb[:], W2[:])
    b1_sb = const.tile([HID, 1], F32)
    nc.gpsimd.dma_start(b1_sb[:], b1[:])
    b2_sb = const.tile([LAT, 1], F32)
    nc.gpsimd.dma_start(b2_sb[:], b2[:])
    id_sb = const.tile([P, P], F32)
    nc.gpsimd.dma_start(id_sb[:], ident[:])
    ip2_sb = const.tile([P, SG * LAT], F32)
    nc.gpsimd.dma_start(ip2_sb[:], ipat_full[:])
    ts_sb = const.tile([1, N_TIME], F32)
    nc.gpsimd.dma_start(ts_sb[:], ts[:])

    ones_sb = const.tile([1, P], F32)
    nc.vector.memset(ones_sb[:], 1.0)
    hrow = const.tile([1, N_TIME], F32)
    nc.vector.tensor_scalar_mul(hrow[:], ts_sb[:], 0.5)

    # Hcol[p, i] = h_i on all 128 partitions (K=1 matmul broadcast).
    # All stage-1 PSUM tiles share one slot (tag s1) so the pool stays at
    # 1 bank and never needs releasing (no released-zone deps downstream).
    s1psum = ctx.enter_context(tc.tile_pool(name="s1psum", bufs=1, space="PSUM"))
    psH = s1psum.tile([P, 512], F32, tag="s1")
    nc.tensor.matmul(psH[:, 0:N_TIME], ones_sb[:], hrow[:], start=True, stop=True)
    Hcol = const.tile([P, N_TIME], F32)
    nc.vector.tensor_copy(Hcol[:], psH[:, 0:N_TIME])

    # rhsD[:, g*512 + s*64 + l'] : rows 0:64 = h_{8g+s}*I64, rows 64:128 = I64.
    rhsD = const.tile([P, NG * SG * LAT], F32)
    for g in range(NG):
        sl = slice(g * SG * LAT, (g + 1) * SG * LAT)
        nc.vector.tensor_mul(
            rhsD[0:LAT, sl].rearrange("p (s l) -> p s l", l=LAT),
            ip2_sb[0:LAT, :].rearrange("p (s l) -> p s l", l=LAT),
            Hcol[0:LAT, g * SG : (g + 1) * SG][:, :, None].broadcast_to(
                [LAT, SG, LAT]
            ),
        )
        nc.vector.tensor_copy(rhsD[LAT:P, sl], ip2_sb[LAT:P, :])

    # ---- active mask: activeF[p, c*64 + i] = 1.0 if any(mask[c*128+p, i, :]) else 0.
    activeF = const.tile([P, NCH * N_TIME], F32)
    MBUFS = 3
    mpool = ctx.enter_context(tc.tile_pool(name="mask", bufs=MBUFS))
    redp = ctx.enter_context(tc.tile_pool(name="red", bufs=2))
    red_insts = []
    for c in range(NCH):
        if c >= MBUFS:
            # Pool-engine nop carrying the DVE wait for the mt slot being
            # reused, so the DMA itself keeps a single (queue) sync wait.
            nop = nc.gpsimd.engine_nop()
            _add_dep(nop.ins, red_insts[c - MBUFS].ins, sync=True,
                     reason="mask slot reuse pre-wait")
        mt = mpool.tile([P, N_TIME * DIM], I32, tag="mt")
        nc.gpsimd.dma_start(mt[:], mask[c * P : (c + 1) * P, :])
        msum = redp.tile([P, N_TIME], I32, tag="msum")
        red_insts.append(nc.vector.tensor_reduce(
            msum[:],
            mt[:].rearrange("p (t d) -> p t d", d=DIM),
            axis=mybir.AxisListType.X,
            op=OP.max,
        ))
        nc.vector.tensor_scalar(
            activeF[:, c * N_TIME : (c + 1) * N_TIME],
            msum[:],
            0,
            None,
            op0=OP.is_gt,
        )

    # ---- stage 1: y0^T, g0 = f(y0), Cb = g0 + b2 (feature-major).
    # Y0T/G0T: [64, T] z-matmul rhs operands (base partition 0).
    # CY rows 0:64 = Cb^T, rows 64:128 = y0^T     (y-matmul lhsT stack)
    Y0T = const.tile([LAT, T], F32)
    G0T = const.tile([LAT, T], F32)
    CY = const.tile([P, T], F32)
    y0p = ctx.enter_context(tc.tile_pool(name="y0p", bufs=NCH))
    for c in range(NCH):
        # yt2 = [fp_chunk | fp_chunk]; its PE transpose lands y0^T twice,
        # at partitions 0:64 (-> Y0T) and 64:128 (-> CY bottom, base-64
        # aligned so no partition-shifting copy is needed).
        yt2 = y0p.tile([P, 2 * LAT], F32, tag="yt2")
        nc.gpsimd.dma_start(yt2[:, 0:LAT], fp[c * P : (c + 1) * P, :])
        nc.gpsimd.dma_start(yt2[:, LAT : 2 * LAT], fp[c * P : (c + 1) * P, :])
        psT = s1psum.tile([P, 512], F32, tag="s1")
        nc.tensor.transpose(psT[:, 0:P], yt2[:], id_sb[:])
        nc.vector.tensor_copy(Y0T[:, c * P : (c + 1) * P], psT[0:LAT, 0:P])
        nc.vector.tensor_copy(CY[LAT:P, c * P : (c + 1) * P], psT[LAT:P, 0:P])

    b2x2 = const.tile([LAT, 1], F32)
    nc.vector.tensor_scalar_mul(b2x2[:], b2_sb[:], 2.0)

    for hlf in range(2):
        sl = slice(hlf * 512, (hlf + 1) * 512)
        psZ0 = s1psum.tile([P, 512], F32, tag="s1")
        nc.tensor.matmul(psZ0[:], W1_sb[:], Y0T[:, sl], start=True, stop=True)
        u0 = y0p.tile([P, 512], F32, tag="u0")
        nc.scalar.activation(u0[:], psZ0[:], AF.Tanh, bias=b1_sb[:, 0:1])
        psG0 = s1psum.tile([LAT, 512], F32, tag="s1")
        nc.tensor.matmul(psG0[:], W2_sb[:], u0[:], start=True, stop=True)
        nc.scalar.activation(G0T[:, sl], psG0[:], AF.Identity, bias=b2_sb[:, 0:1])
        nc.scalar.activation(CY[0:LAT, sl], psG0[:], AF.Identity, bias=b2x2[:, 0:1])

    # ---- main loop over step groups.
    wpool = ctx.enter_context(tc.tile_pool(name="wpool", bufs=2 * SG + 2))
    zpool = ctx.enter_context(tc.tile_pool(name="zpool", bufs=2, space="PSUM"))
    upool = ctx.enter_context(tc.tile_pool(name="upool", bufs=SG + 2))
    ypool = ctx.enter_context(tc.tile_pool(name="ypool", bufs=3, space="PSUM"))
    opool = ctx.enter_context(tc.tile_pool(name="opool", bufs=3))

    for g in range(NG):
        uts = []
        w2s = []
        for s in range(SG):
            i = g * SG + s
            w1s = wpool.tile([LAT, HID], F32, tag="w1s")
            nc.vector.tensor_scalar_mul(w1s[:], W1_sb[:], Hcol[0:LAT, i : i + 1])
            w2si = wpool.tile([HID, LAT], F32, tag="w2s")
            nc.vector.tensor_scalar_mul(w2si[:], W2_sb[:], Hcol[:, i : i + 1])
            w2s.append(w2si)

            psZ = zpool.tile([P, 1024], F32, tag="psZ")
            for hlf in range(2):
                sl = slice(hlf * 512, (hlf + 1) * 512)
                nc.tensor.matmul(
                    psZ[:, sl], W1_sb[:], Y0T[:, sl], start=True, stop=False
                )
                nc.tensor.matmul(
                    psZ[:, sl], w1s[:], G0T[:, sl], start=False, stop=True
                )
            ut = upool.tile([P, 1024], F32, tag="ut")
            nc.scalar.activation(ut[:], psZ[:], AF.Tanh, bias=b1_sb[:, 0:1])
            uts.append(ut)

        for c in range(NCH):
            csl = slice(c * P, (c + 1) * P)
            psY = ypool.tile([P, SG * LAT], F32, tag="psY")
            nc.tensor.matmul(
                psY[:],
                CY[:, csl],
                rhsD[:, g * SG * LAT : (g + 1) * SG * LAT],
                start=True,
                stop=False,
            )
            for s in range(SG):
                nc.tensor.matmul(
                    psY[:, s * LAT : (s + 1) * LAT],
                    uts[s][:, csl],
                    w2s[s][:],
                    start=False,
                    stop=(s == SG - 1),
                    skip_group_check=True,
                )
            ot = opool.tile([P, SG * LAT], F32, tag="ot")
            drain = nc.vector.scalar_tensor_tensor(
                ot[:].rearrange("p (s l) -> p s l", l=LAT),
                psY[:].rearrange("p (s l) -> p s l", l=LAT),
                1.0,
                activeF[:, c * N_TIME + g * SG : c * N_TIME + (g + 1) * SG][
                    :, :, None
                ].broadcast_to([P, SG, LAT]),
                op0=OP.bypass,
                op1=OP.mult,
            )
            nop = nc.gpsimd.engine_nop()
            _add_dep(nop.ins, drain.ins, sync=True, reason="out pre-wait")
            nc.gpsimd.dma_start(
                out[c * P : (c + 1) * P, g * SG * LAT : (g + 1) * SG * LAT], ot[:]
            )


def _build():
    if "nc" in _cache:
        return _cache["nc"]
    nc = bacc.Bacc("TRN2", target_bir_lowering=False, debug=False)
    fp = nc.dram_tensor("fp", [T, LAT], F32, kind="ExternalInput")
    ts = nc.dram_tensor("ts", [1, N_TIME], F32, kind="ExternalInput")
    mask = nc.dram_tensor("mask", [T, N_TIME * DIM], I32, kind="ExternalInput")
    W1 = nc.dram_tensor("W1", [LAT, HID], F32, kind="ExternalInput")
    b1 = nc.dram_tensor("b1", [HID, 1], F32, kind="ExternalInput")
    W2 = nc.dram_tensor("W2", [HID, LAT], F32, kind="ExternalInput")
    b2 = nc.dram_tensor("b2", [LAT, 1], F32, kind="ExternalInput")
    out = nc.dram_tensor("out", [T, N_TIME * LAT], F32, kind="ExternalOutput")

    ident = nc.inline_tensor(np.eye(128, dtype=np.float32), name="ident")
    ipat_full = nc.inline_tensor(
        np.tile(np.eye(LAT, dtype=np.float32), (2, SG)), name="ipat_full"
    )

    with tile.TileContext(nc) as tc:
        with ExitStack() as ctx:
            _emit(ctx, tc, nc, fp, ts, mask, W1, b1, W2, b2, out, ident, ipat_full)
    _strip_same_engine_waits(nc)
    nc.compile()
    _cache["nc"] = nc
    return nc


def _strip_same_engine_waits(nc):
    """Drop sem waits that target the instruction's own engine proc sem.

    Each engine executes its instruction stream in order and its proc sem is
    only incremented by its own completed instructions, so a wait on your own
    engine's sem is satisfied by construction. Tile emits these conservatively
    (it doesn't track transitive same-proc knowledge); walrus codegen caps
    instructions at 2 sync waits, so the redundant ones must go.
    """
    eng_prefix = {
        "PE": "PE_",
        "DVE": "DVE_",
        "Activation": "Activation_",
        "SP": "SP_",
        "Pool": "Pool_",
    }
    for fn in nc.m.functions:
        for blk in fn.blocks:
            for inst in blk.instructions:
                si = getattr(inst, "sync_info", None)
                if si is None or not si.on_wait:
                    continue
                eng = getattr(inst, "engine", None)
                pref = eng_prefix.get(getattr(eng, "value", None) or str(eng), None)
                if pref is None:
                    continue
                kept = [
                    w
                    for w in si.on_wait
                    if not (getattr(w, "ant_name", "") or "").startswith(pref)
                ]
                if len(kept) != len(si.on_wait):
                    si.on_wait = kept


def kernel(first_point, time_steps, mask, W1, b1, W2, b2, trace=False, **trace_kw):
    first_point = np.asarray(first_point)
    time_steps = np.asarray(time_steps)
    mask = np.asarray(mask)
    W1a = np.ascontiguousarray(np.asarray(W1), dtype=np.float32)
    b1a = np.ascontiguousarray(np.asarray(b1), dtype=np.float32).reshape(HID, 1)
    W2a = np.ascontiguousarray(np.asarray(W2), dtype=np.float32)
    b2a = np.ascontiguousarray(np.asarray(b2), dtype=np.float32).reshape(LAT, 1)
    tsa = np.ascontiguousarray(time_steps, dtype=np.float32).reshape(1, N_TIME)

    fp_full = np.ascontiguousarray(first_point[0], dtype=np.float32)  # [8192, 64]
    mask_full = np.ascontiguousarray(mask, dtype=np.int32).reshape(
        N_TRAJ, N_TIME * DIM
    )

    nc = _build()
    in_maps = []
    for c in range(NCORES):
        sl = slice(c * T, (c + 1) * T)
        in_maps.append(
            {
                "fp": np.ascontiguousarray(fp_full[sl]),
                "ts": tsa,
                "mask": np.ascontiguousarray(mask_full[sl]),
                "W1": W1a,
                "b1": b1a,
                "W2": W2a,
                "b2": b2a,
            }
        )

    res = run_bass_kernel_spmd(
        nc, in_maps, core_ids=list(range(NCORES)), trace=trace, **trace_kw
    )
    outs = [r["out"].reshape(T, N_TIME, 1, LAT) for r in res.results]
    full = np.concatenate(outs, axis=0)
    if trace:
        kernel.last_result = res
    return full



# revision 35
# speedup vs baseline: 3.7054x; 3.7054x over previous
"""Trainium2 Bass kernel for nn_DiffeqSolver (two-step Euler MLP-ODE).

Math (per trajectory n, time step i):
    f(y) = tanh(y@W1 + b1)@W2 + b2
    h_i = t_i / 2
    y1_i = y0 + h_i*f(y0)
    y2_i = y1_i + h_i*f(y1_i)
    out[n,i,:] = active[n,i] ? y2_i : 0      (active = any(mask[n,i,:] > 0))
    (t_i == 0 gives y2 == y0 exactly, so the reference's pos-branch is folded.)

Device refactor (feature-major per step, traj-major assembled by PE transpose):
    g0  = f(y0), Cb = g0 + b2                                    (stage 1)
    z_i = [W1; h_i W1]^T [y0^T; g0^T] + b1     (one stacked fp32r matmul)
    u_i = tanh(z_i)                            (ACT)
    v_i = (h_i W2)^T u_i                       (fp32r matmul, pair-packed PSUM)
    y2 chunk (traj-major) = CY^T rhsD  (+)  PE-transpose of bf16 v pairs
      where CY = [Cb; y0] (bf16), rhsD packs [h_i I64; I64] (bf16)
    drain applies the active-mask (DVE/Pool) -> contiguous [128, 8x64] DMA out.

Engine split: PE matmuls; ACT tanh + half the pair drains + const DMAs;
DVE prep + half pair drains + most mask drains; Pool mask reduce + rest;
SP issues all mask-in / result-out DMAs (HWDGE).

Sharding: data-parallel over trajectories, 1024 per core x 8 cores.
"""

import numpy as np
import ml_dtypes
from contextlib import ExitStack

import concourse.bass as bass
import concourse.bacc as bacc
import concourse.mybir as mybir
import concourse.tile as tile
from concourse.bass_utils import run_bass_kernel_spmd

N_TRAJ, N_TIME, LAT, HID, DIM = 8192, 64, 64, 128, 64
NCORES = 8
T = N_TRAJ // NCORES          # 1024 trajectories per core
NCH = T // 128                # 8 chunks of 128 trajectories
SG = 8                        # steps per group
NG = N_TIME // SG             # 8 step groups
P = 128
F32 = mybir.dt.float32
F32R = mybir.dt.float32r
BF16 = mybir.dt.bfloat16
I32 = mybir.dt.int32
AF = mybir.ActivationFunctionType
OP = mybir.AluOpType

_cache = {}


def _emit(ctx, tc, nc, fp, ts, mask, W1, b1, W2, b2, out, ident, ipat2):
    const = ctx.enter_context(tc.tile_pool(name="const", bufs=1))

    # ---- const DMAs (ACT queue, HWDGE). W1 twice -> stacked [W1; W1].
    id_sb = const.tile([P, P], F32)
    nc.scalar.dma_start(id_sb[:], ident[:])
    ts_sb = const.tile([1, N_TIME], F32)
    nc.scalar.dma_start(ts_sb[:], ts[:])
    W1x2 = const.tile([P, HID], F32)
    nc.scalar.dma_start(W1x2[0:LAT, :], W1[:])
    nc.scalar.dma_start(W1x2[LAT:P, :], W1[:])
    W2_sb = const.tile([HID, LAT], F32)
    nc.scalar.dma_start(W2_sb[:], W2[:])
    b1_sb = const.tile([HID, 1], F32)
    nc.scalar.dma_start(b1_sb[:], b1[:])
    b2b = const.tile([P, 1], F32)
    nc.scalar.dma_start(b2b[0:LAT, :], b2[:])
    nc.scalar.dma_start(b2b[LAT:P, :], b2[:])
    # rhsD + stage-1 yt2 loads go on the SP queue: HWDGE completion sems are
    # 8 lanes round-robin over ALL non-Pool DMAs in emission order, so every
    # fast DMA a compute op waits on must stay clear of slow transfers.
    rhsD = const.tile([P, NG * SG * LAT], BF16)
    nc.sync.dma_start(rhsD[:], ipat2[:])

    # ---- h vectors: hrow [1,64]; Hcol[p,i] = h_i on all 128 partitions.
    ones_sb = const.tile([1, P], F32)
    nc.vector.memset(ones_sb[:], 1.0)
    hrow = const.tile([1, N_TIME], F32)
    nc.vector.tensor_scalar_mul(hrow[:], ts_sb[:], 0.5)

    ypsum = ctx.enter_context(tc.tile_pool(name="ypsum", bufs=3, space="PSUM"))
    psH = ypsum.tile([P, 512], F32, tag="psY")
    nc.tensor.matmul(psH[:, 0:N_TIME], ones_sb[:], hrow[:], start=True, stop=True)
    Hcol = const.tile([P, N_TIME], F32)
    nc.vector.tensor_copy(Hcol[:], psH[:, 0:N_TIME])

    # scale2: rows 0:64 = 1.0, rows 64:128 = h_i  (per-step column scaling
    # for the stacked [W1; h W1] lhsT build).
    scale2 = const.tile([P, N_TIME], F32)
    nc.vector.memset(scale2[0:LAT, :], 1.0)
    nc.vector.tensor_copy(scale2[LAT:P, :], Hcol[LAT:P, :])

    # rhsD upper rows: h_i * I64 (in-place over the DMA'd identity pattern).
    nc.vector.tensor_mul(
        rhsD[0:LAT, :].rearrange("p (i l) -> p i l", l=LAT),
        rhsD[0:LAT, :].rearrange("p (i l) -> p i l", l=LAT),
        Hcol[0:LAT, :][:, :, None].broadcast_to([LAT, N_TIME, LAT]),
    )

    # ---- stage 1: YG = [y0^T; g0^T] (f32), CY = [Cb^T; y0^T] (bf16).
    # fp loaded twice into doubled per-chunk columns [fp_c | fp_c]: the PE
    # transpose of each [128, 128] doubled block lands y0^T at partitions
    # 0:64 (-> YG top) AND 64:128 (-> CY bottom); transpose outputs must
    # start at PSUM partition 0, so the doubling does the partition shift.
    YG = const.tile([P, T], BF16)
    CY = const.tile([P, T], BF16)
    W1x2b = const.tile([P, HID], BF16)
    nc.vector.tensor_copy(W1x2b[:], W1x2[:])
    W2b = const.tile([HID, LAT], BF16)
    nc.vector.tensor_copy(W2b[:], W2_sb[:])
    fpall2 = const.tile([P, NCH * 2 * LAT], F32)
    for rep in range(2):
        nc.sync.dma_start(
            fpall2[:].rearrange("p (c rl) -> p c rl", rl=2 * LAT)[
                :, :, rep * LAT : (rep + 1) * LAT
            ],
            fp[:].rearrange("(c p) l -> p c l", p=P),
        )
    y0p = ctx.enter_context(tc.tile_pool(name="y0p", bufs=3))
    for c in range(NCH):
        psT = ypsum.tile([P, 512], F32, tag="psY")
        nc.tensor.transpose(
            psT[:, 0:P], fpall2[:, c * 2 * LAT : (c + 1) * 2 * LAT], id_sb[:]
        )
        nc.vector.tensor_copy(YG[0:LAT, c * P : (c + 1) * P], psT[0:LAT, 0:P])
        nc.vector.tensor_copy(CY[LAT:P, c * P : (c + 1) * P], psT[LAT:P, 0:P])

    b2x2 = const.tile([LAT, 1], F32)
    nc.scalar.mul(b2x2[:], b2b[0:LAT, :], 2.0)

    for hlf in range(2):
        sl = slice(hlf * 512, (hlf + 1) * 512)
        psA = ypsum.tile([P, 512], F32, tag="psY")
        nc.tensor.matmul(
            psA[:], W1x2b[0:LAT, :], YG[0:LAT, sl],
            start=True, stop=True,
        )
        u0 = y0p.tile([P, 512], BF16, tag="u0")
        nc.scalar.activation(u0[:], psA[:], AF.Tanh, bias=b1_sb[:, 0:1])
        # g0^T into YG rows 64:128 (matmul lands there directly), and the
        # same product at rows 0:64 for CY's Cb = g0 + b2 (+b2 again).
        psGh = ypsum.tile([P, 512], F32, tag="psY")
        nc.tensor.matmul(
            psGh[LAT:P, :], W2b[:], u0[:],
            start=True, stop=True,
        )
        nc.scalar.activation(
            YG[LAT:P, sl], psGh[LAT:P, :], AF.Identity, bias=b2b[LAT:P, 0:1]
        )
        psGl = ypsum.tile([P, 512], F32, tag="psY")
        nc.tensor.matmul(
            psGl[0:LAT, :], W2b[:], u0[:],
            start=True, stop=True,
        )
        nc.scalar.activation(
            CY[0:LAT, sl], psGl[0:LAT, :], AF.Identity, bias=b2x2[:, 0:1]
        )

    # ---- mask DMAs (SP queue, emitted after every stage-1 DMA so those stay
    # on clean DMAHW lanes). "any(mask)>0" runs entirely on DVE (Pool's
    # walrus codegen rejects int max / TensorScalarPtr): tensor_reduce max +
    # is_gt per chunk, spliced into the DVE stream at slots that track the
    # serial mask-DMA arrival (~5.8us per chunk) so DVE never parks early.
    activeF = const.tile([P, NCH * N_TIME], F32)
    MBUFS = 4
    mpool = ctx.enter_context(tc.tile_pool(name="mask", bufs=MBUFS))
    redp = ctx.enter_context(tc.tile_pool(name="red", bufs=2))
    mts = {}
    for c in range(NCH):
        mt = mpool.tile([P, N_TIME * DIM], I32, tag="mt")
        nc.sync.dma_start(mt[:], mask[c * P : (c + 1) * P, :])
        mts[c] = mt

    def emit_dve_reduce(c):
        msum = redp.tile([P, N_TIME], I32, tag="msum")
        nc.vector.tensor_reduce(
            msum[:],
            mts[c][:].rearrange("p (t d) -> p t d", d=DIM),
            axis=mybir.AxisListType.X,
            op=OP.max,
        )
        nc.vector.tensor_scalar(
            activeF[:, c * N_TIME : (c + 1) * N_TIME],
            msum[:],
            0,
            None,
            op0=OP.is_gt,
        )

    emit_dve_reduce(0)
    # chunk c's reduce is spliced into the main loop at REDUCE_SLOT[c]
    REDUCE_SLOTS = {
        (0, 5): 1, (1, 3): 2, (2, 0): 3, (2, 7): 4,
        (3, 5): 5, (4, 3): 6, (5, 1): 7,
    }

    # ---- main loop.
    # Drain scheduling: block (g, c) is "late" when its activeF(c) (paced by
    # the serial mask-DMA + Pool tree, ~6 us per chunk) lands after the block's
    # natural emission slot. Late blocks drain in two phases so no engine
    # stream ever parks on a late activeF: DVE does a plain PSUM->SBUF copy at
    # the natural slot, Pool applies the mask in-place (SBUF only) at a
    # deferred slot, and the out-DMA goes with phase B.
    wpool = ctx.enter_context(tc.tile_pool(name="wpool", bufs=2))
    zpsum = ctx.enter_context(tc.tile_pool(name="zpsum", bufs=2, space="PSUM"))
    upool = ctx.enter_context(tc.tile_pool(name="upool", bufs=16))
    opool = ctx.enter_context(tc.tile_pool(name="opool", bufs=16))
    olpool = ctx.enter_context(tc.tile_pool(name="olpool", bufs=16))

    uts = [None] * (NG * SG)
    w2hs = [None] * NG
    late_ot = {}

    def is_late(gg, c):
        return c >= gg + 2

    slotB = {}
    for gg in range(NG):
        for c in range(NCH):
            if is_late(gg, c):
                slotB.setdefault((max(gg + 2, c - 1), gg), []).append((gg, c))

    def emit_w_group(g):
        # stacked lhsT [W1; h W1] for the group's 8 steps, and h*W2 (bf16).
        # Built on Pool (float multiply is legal there) to keep DVE free for
        # the PSUM drains + mask reduces.
        W1S = wpool.tile([P, SG * HID], BF16, tag="w1s")
        nc.gpsimd.tensor_mul(
            W1S[:].rearrange("p (s k) -> p s k", k=HID),
            W1x2[:][:, None, :].broadcast_to([P, SG, HID]),
            scale2[:, g * SG : (g + 1) * SG][:, :, None].broadcast_to([P, SG, HID]),
        )
        w2h = wpool.tile([HID, SG * LAT], BF16, tag="w2s", bufs=3)
        nc.gpsimd.tensor_mul(
            w2h[:].rearrange("p (s l) -> p s l", l=LAT),
            W2_sb[:][:, None, :].broadcast_to([HID, SG, LAT]),
            Hcol[:, g * SG : (g + 1) * SG][:, :, None].broadcast_to([HID, SG, LAT]),
        )
        return W1S, w2h

    def active_bc(gg, c):
        return activeF[:, c * N_TIME + gg * SG : c * N_TIME + (gg + 1) * SG][
            :, :, None
        ].broadcast_to([P, SG, LAT])

    def emit_phaseA(gg, c):
        """psY = y0 + h*Cb (CY@rhsD) + per-step bf16 u^T (h W2); then either a
        masked DVE drain (activeF ready by now) or a plain copy for phase B."""
        csl = slice(c * P, (c + 1) * P)
        w2h = w2hs[gg]
        psY = ypsum.tile([P, SG * LAT], F32, tag="psY")
        nc.tensor.matmul(
            psY[:], CY[:, csl], rhsD[:, gg * SG * LAT : (gg + 1) * SG * LAT],
            start=True, stop=False,
        )
        for s in range(SG):
            # the s==7 matmul carries the group stop and must NOT skip the
            # group check: skip_group_check bypasses group tracking entirely
            # (including the stop), which would leave the bank flagged open.
            nc.tensor.matmul(
                psY[:, s * LAT : (s + 1) * LAT],
                uts[gg * SG + s][:, csl],
                w2h[:, s * LAT : (s + 1) * LAT],
                start=False,
                stop=(s == SG - 1),
                skip_group_check=(s != SG - 1),
            )
        if is_late(gg, c):
            otl = olpool.tile([P, SG * LAT], F32, tag="otl")
            nc.scalar.activation(otl[:], psY[:], AF.Identity)
            late_ot[(gg, c)] = otl
        else:
            ot = opool.tile([P, SG * LAT], F32, tag="ot")
            nc.vector.scalar_tensor_tensor(
                ot[:].rearrange("p (s l) -> p s l", l=LAT),
                psY[:].rearrange("p (s l) -> p s l", l=LAT),
                1.0,
                active_bc(gg, c),
                op0=OP.bypass,
                op1=OP.mult,
            )
            nc.sync.dma_start(
                out[c * P : (c + 1) * P, gg * SG * LAT : (gg + 1) * SG * LAT],
                ot[:],
            )

    def emit_phaseB(gg, c):
        otl = late_ot.pop((gg, c))
        nc.gpsimd.tensor_mul(
            otl[:].rearrange("p (s l) -> p s l", l=LAT),
            otl[:].rearrange("p (s l) -> p s l", l=LAT),
            active_bc(gg, c),
        )
        nc.sync.dma_start(
            out[c * P : (c + 1) * P, gg * SG * LAT : (gg + 1) * SG * LAT], otl[:]
        )

    W1S_cur, w2hs[0] = emit_w_group(0)
    W1S_next = None

    for g in range(NG):
        for s in range(SG):
            psZ = zpsum.tile([P, 2 * 512], F32, tag="psZ")
            for hlf in range(2):
                sl = slice(hlf * 512, (hlf + 1) * 512)
                nc.tensor.matmul(
                    psZ[:, sl],
                    W1S_cur[:, s * HID : (s + 1) * HID],
                    YG[:, sl],
                    start=True, stop=True, skip_group_check=True,
                )
            ut = upool.tile([P, T], BF16, tag="ut")
            nc.scalar.activation(ut[:], psZ[:], AF.Tanh, bias=b1_sb[:, 0:1])
            uts[g * SG + s] = ut

            if s == 0 and g + 1 < NG:
                # prefetch next group's step weights on DVE
                W1S_next, w2h_next = emit_w_group(g + 1)
                w2hs[g + 1] = w2h_next

            if (g, s) in REDUCE_SLOTS:
                emit_dve_reduce(REDUCE_SLOTS[(g, s)])

            for b in slotB.get((g, s), []):
                emit_phaseB(*b)
            if g >= 1:
                emit_phaseA(g - 1, s)
        if g + 1 < NG:
            W1S_cur = W1S_next

    for c in range(NCH):
        emit_phaseA(NG - 1, c)
    for key in sorted(k for k in slotB if k[0] >= NG):
        for b in slotB[key]:
            emit_phaseB(*b)


def _build(strip=True):
    if "nc" in _cache:
        return _cache["nc"]
    nc = bacc.Bacc("TRN2", target_bir_lowering=False, debug=False)
    fp = nc.dram_tensor("fp", [T, LAT], F32, kind="ExternalInput")
    ts = nc.dram_tensor("ts", [1, N_TIME], F32, kind="ExternalInput")
    mask = nc.dram_tensor("mask", [T, N_TIME * DIM], I32, kind="ExternalInput")
    W1 = nc.dram_tensor("W1", [LAT, HID], F32, kind="ExternalInput")
    b1 = nc.dram_tensor("b1", [HID, 1], F32, kind="ExternalInput")
    W2 = nc.dram_tensor("W2", [HID, LAT], F32, kind="ExternalInput")
    b2 = nc.dram_tensor("b2", [LAT, 1], F32, kind="ExternalInput")
    out = nc.dram_tensor("out", [T, N_TIME * LAT], F32, kind="ExternalOutput")

    ident = nc.inline_tensor(np.eye(128, dtype=np.float32), name="ident")
    ipat2 = nc.inline_tensor(
        np.tile(np.eye(LAT, dtype=np.float32), (2, N_TIME)).astype(
            ml_dtypes.bfloat16
        ),
        name="ipat2",
    )

    with tile.TileContext(nc) as tc:
        with ExitStack() as ctx:
            _emit(ctx, tc, nc, fp, ts, mask, W1, b1, W2, b2, out, ident, ipat2)
    if strip:
        _strip_same_engine_waits(nc)
    nc.compile()
    _cache["nc"] = nc
    return nc


def _strip_same_engine_waits(nc):
    """Drop sem waits that target the instruction's own engine proc sem.

    Each engine executes its instruction stream in order and its proc sem is
    only incremented by its own completed instructions, so a wait on your own
    engine's sem is satisfied by construction. Tile emits these conservatively
    (it doesn't track transitive same-proc knowledge); walrus codegen caps
    instructions at 2 sync waits, so the redundant ones must go.
    """
    eng_prefix = {
        "PE": "PE_",
        "DVE": "DVE_",
        "Activation": "Activation_",
        "SP": "SP_",
        "Pool": "Pool_",
    }
    for fn in nc.m.functions:
        for blk in fn.blocks:
            for inst in blk.instructions:
                si = getattr(inst, "sync_info", None)
                if si is None or not si.on_wait:
                    continue
                eng = getattr(inst, "engine", None)
                pref = eng_prefix.get(getattr(eng, "value", None) or str(eng), None)
                if pref is None:
                    continue
                kept = [
                    w
                    for w in si.on_wait
                    if not (getattr(w, "ant_name", "") or "").startswith(pref)
                ]
                if len(kept) != len(si.on_wait):
                    si.on_wait = kept


def kernel(first_point, time_steps, mask, W1, b1, W2, b2, trace=False, **trace_kw):
    first_point = np.asarray(first_point)
    time_steps = np.asarray(time_steps)
    mask = np.asarray(mask)
    W1a = np.ascontiguousarray(np.asarray(W1), dtype=np.float32)
    b1a = np.ascontiguousarray(np.asarray(b1), dtype=np.float32).reshape(HID, 1)
    W2a = np.ascontiguousarray(np.asarray(W2), dtype=np.float32)
    b2a = np.ascontiguousarray(np.asarray(b2), dtype=np.float32).reshape(LAT, 1)
    tsa = np.ascontiguousarray(time_steps, dtype=np.float32).reshape(1, N_TIME)

    fp_full = np.ascontiguousarray(first_point[0], dtype=np.float32)  # [8192, 64]
    mask_full = np.ascontiguousarray(mask, dtype=np.int32).reshape(
        N_TRAJ, N_TIME * DIM
    )

    nc = _build()
    in_maps = []
    for c in range(NCORES):
        sl = slice(c * T, (c + 1) * T)
        in_maps.append(
            {
                "fp": np.ascontiguousarray(fp_full[sl]),
                "ts": tsa,
                "mask": np.ascontiguousarray(mask_full[sl]),
                "W1": W1a,
                "b1": b1a,
                "W2": W2a,
                "b2": b2a,
            }
        )

    res = run_bass_kernel_spmd(
        nc, in_maps, core_ids=list(range(NCORES)), trace=trace, **trace_kw
    )
    outs = [r["out"].reshape(T, N_TIME, 1, LAT) for r in res.results]
    full = np.concatenate(outs, axis=0)
    if trace:
        kernel.last_result = res
    return full
